# revision 31
# baseline (speedup 1.0000x reference)
"""Trainium2 Bass kernel for the Brill-Lindquist Christoffel-symbol grid.

Math: the reference reduces to
    psi  = 1 + sum_n m_n / (2 r_n),   m = softplus(pre)
    h    = psi^4
    G_c  = finite-difference gradient of h along grid axis c (2nd order
           central interior, 1st order one-sided edges, spacing DX)
    W_c  = 0.5 * G_c / h
    Gamma^i_{jk} = delta_ij W_k + delta_ik W_j - delta_jk W_i
so the [96,96,96,3,3,3] output is +-W_c scattered over 27 slots per point.

Sharding: axis 0 (12 planes per core x 8 cores); h is analytic in the
inputs so each core evaluates its slab + 1-tile halo directly. Rows are
packed row = a0*96+a1 (9 tiles of 128 partitions), free dim = a2.

Design (output-DMA roofline ~29 us/core + prologue/fill):
  - h chain: s_n = r_n/mh_n via host-prescaled crow/ab broadcast adds
    (Pool+DVE into one [.,2W] tile), ONE Act sqrt over both BHs, ONE
    DVE reciprocal, psi-1 = sinv_lo+sinv_hi (Pool), hsq=(psi)^2 (Act,
    bias=1), h = hsq^2 bf16 (Pool); 1/h hoisted per-chunk (DVE recip +
    Pool square).
  - axis-0/1 FD: 6 bf16 matmuls/tile (fp32 psum), single-bf16 h
    (rel err 2.9e-3, budget 2e-2). z-FD via shifted subtract (Pool),
    kvec scale folded before the 1/h product.
  - scatter via the (z,c)-interleaved w3 row into the 27-slot output:
    9 diagonal slots in one packed-inner Act op; +/- pairs split
    DVE/Act at their measured-efficient op shapes (strided writes cost
    ~1.5-2 ns/elem on DVE/Act and are catastrophic on GpSimd, which
    also cannot read PSUM).
  - just-in-time h chunks emitted one tile-group ahead of their
    consumers; blocks 0-2 and tile 0 computed in z-halves so the first
    output DMA issues ~10 us earlier; PE pstate-ramp warmup matmuls;
    Act sqrt-table preload; 5 rotating output buffers; debug=False.
"""

import numpy as np

RES = 96
N_CORES = 8
PLANES = RES // N_CORES        # 12
LROWS = PLANES * RES           # 1152 local rows
NT = LROWS // 128              # 9 local 128-row tiles
EXTNT = NT + 2                 # 11 extended tiles (halo)
NROWS_G = RES * RES            # 9216 global rows
S27 = 27
NOB = 5                        # rotating output buffers

# misc input tile [128, MW] column layout (host-prescaled by 1/mh^2)
M_CROW = 0        # (z-pz_n)^2/mh_n^2 [96] x2
M_KVEC = 192      # z-FD column scale [96] (0.25/DX interior, 0.5/DX edge)
M_AB = 288        # ((x-px_n)^2+(y-py_n)^2)/mh_n^2 [11] x2
MW = 312

# dmat entry order: tile-0's two entries first so a small leading DMA
# unblocks the first tile's matmuls early.
ORDER = [0, 3, 1, 4, 5, 2]
SLOT = {e: i for i, e in enumerate(ORDER)}

# h-phase chunks (ext-block ranges): singles first for fill latency
CHUNKS = [(0, 1), (1, 2), (2, 3), (3, 5), (5, 7), (7, 9), (9, 11)]
TILES_AFTER_CHUNK = {2: [0], 3: [1, 2], 4: [3, 4], 5: [5, 6], 6: [7, 8]}


def _grid_x():
    # Match the reference grid bit-for-bit: jnp.linspace in fp32 on CPU
    # (the reference's softplus cannot compile for the neuron backend, so
    # it necessarily runs on the jax CPU platform).
    import jax
    import jax.numpy as jnp
    MAX_X = 1.0
    DX = np.float32(MAX_X / (RES / 2 - 1))

    def _ls():
        return jnp.linspace(
            DX * (1 - RES / 2), DX * (RES / 2 - 1), RES, dtype=jnp.float32
        )

    try:
        with jax.default_device(jax.devices("cpu")[0]):
            x = np.asarray(_ls())
    except Exception:
        x = np.asarray(_ls())
    return x, float(DX)


def _fd_sources(idx, coeff_c, coeff_e):
    """(offset, coeff) pairs for d/didx with 1st-order one-sided edges."""
    if idx == 0:
        return [(1, coeff_e), (0, -coeff_e)]
    if idx == RES - 1:
        return [(0, coeff_e), (-1, -coeff_e)]
    return [(1, coeff_c), (-1, -coeff_c)]


def _build_dmat(core, DX):
    """[128, 6*3*128] bf16 FD matrices as matmul lhsT ([q, p] = coeff of
    ext-row q in output row p); 0.5 Christoffel factor folded in. All
    values are +-0.25/DX or +-0.5/DX = +-11.75 / +-23.5, exact in bf16.
    Logical entries: 0 g0(t=0), 1 g0(interior), 2 g0(t=8), 3..5 g1(t%3);
    stored in column slots per ORDER."""
    import ml_dtypes
    c0 = 0.5 * (1.0 / (2.0 * np.float64(DX)))
    ce = 0.5 * (1.0 / np.float64(DX))
    out = np.zeros((128, 6 * 3 * 128), np.float64)

    def fill(entry, t, axis):
        slot = SLOT[entry]
        for p in range(128):
            gr = core * LROWS + 128 * t + p
            a = (gr // RES) if axis == 0 else (gr % RES)
            step = RES if axis == 0 else 1
            for off, cf in _fd_sources(a, c0, ce):
                g2 = gr + off * step
                e_ = g2 - core * LROWS + 128
                j = e_ // 128 - t
                q = e_ - 128 * (t + j)
                assert 0 <= j <= 2 and 0 <= q < 128, (core, t, p, off)
                out[q, (slot * 3 + j) * 128 + p] = cf

    fill(0, 0, 0)
    fill(1, 1, 0)
    fill(2, NT - 1, 0)
    for v in range(3):
        fill(3 + v, v, 1)
    return out.astype(ml_dtypes.bfloat16)


def _g0_slot(t):
    return SLOT[0] if t == 0 else (SLOT[2] if t == NT - 1 else SLOT[1])


def _g1_slot(t):
    return SLOT[3 + (t % 3)]


def _build_static(core, x, DX):
    slab = core * LROWS
    e = np.arange(EXTNT * 128)
    g = np.clip(slab - 128 + e, 0, NROWS_G - 1)   # clamp halo overrun (unused rows)
    xcol = x[g % RES].reshape(EXTNT, 128).T.copy()     # X coordinate (a1)
    ycol = x[g // RES].reshape(EXTNT, 128).T.copy()    # Y coordinate (a0)
    kvec = np.full(RES, 0.25 / DX, np.float64)
    kvec[0] = kvec[-1] = 0.5 / DX
    return {
        "xcol": np.ascontiguousarray(xcol, np.float64),
        "ycol": np.ascontiguousarray(ycol, np.float64),
        "kvec": kvec,
        "dmat": _build_dmat(core, DX),
    }


def _build_misc(static, x, pos, mh):
    """Per-core [128, MW] fp32 misc tile: prescaled crow/ab + kvec."""
    misc = np.zeros((128, MW), np.float64)
    for n in range(2):
        crow = (x.astype(np.float64) - pos[n, 2]) ** 2 / (mh[n] * mh[n])
        misc[:, M_CROW + RES * n:M_CROW + RES * (n + 1)] = crow[None, :]
        ab = ((static["xcol"] - pos[n, 0]) ** 2
              + (static["ycol"] - pos[n, 1]) ** 2) / (mh[n] * mh[n])
        misc[:, M_AB + EXTNT * n:M_AB + EXTNT * (n + 1)] = ab
    misc[:, M_KVEC:M_KVEC + RES] = static["kvec"][None, :]
    return np.ascontiguousarray(misc, np.float32)


def _build_program():
    import dataclasses as _dc

    import concourse.bacc as bacc
    import concourse.mybir as mybir
    import concourse.tile as tile

    DT = mybir.dt.float32
    BF = mybir.dt.bfloat16
    AF = mybir.ActivationFunctionType

    nc = bacc.Bacc(None, target_bir_lowering=False, debug=False)
    d_misc = nc.dram_tensor("misc", [128, MW], DT, kind="ExternalInput")
    d_dmat = nc.dram_tensor("dmat", [128, 6 * 3 * 128], BF, kind="ExternalInput")
    d_out = nc.dram_tensor("out", [LROWS, RES * S27], DT, kind="ExternalOutput")

    HW_ = EXTNT * RES             # 1056: free width of the ext h field
    with tile.TileContext(nc) as tc:
        with (
            tc.tile_pool(name="const", bufs=1) as cpool,
            tc.tile_pool(name="work", bufs=4) as wpool,
            tc.tile_pool(name="wout", bufs=4) as wopool,
            tc.tile_pool(name="obuf", bufs=1) as opool,
            tc.tile_pool(name="psum", bufs=3, space="PSUM") as pspool,
            tc.tile_pool(name="psw", bufs=1, space="PSUM") as pswpool,
        ):
            # --- inputs in: misc first (unblocks the h chain), dmat in
            # two pieces (tile-0's entries lead) ---
            mi = cpool.tile([128, MW], DT)
            nc.sync.dma_start(mi[:], d_misc[:])
            dm = cpool.tile([128, 6 * 3 * 128], BF)
            nc.sync.dma_start(dm[:, :6 * 128], d_dmat[:, :6 * 128])
            nc.sync.dma_start(dm[:, 6 * 128:], d_dmat[:, 6 * 128:])

            # --- persistent fields ---
            HSQ = cpool.tile([128, HW_], DT)   # psi^2, fp32
            Hb = cpool.tile([128, HW_], BF)    # h = psi^4, bf16 (FD source)
            HINV = cpool.tile([128, HW_], DT)  # 1/h

            # --- warmups: Act sqrt-table preload (dep-free garbage read)
            # + PE pstate ramp, both overlap the input DMAs ---
            jact = cpool.tile([1, 8], DT, tag="jact")
            nc.vector.memset(jact[:], 1.0)
            nc.scalar.activation(jact[:], jact[:], AF.Sqrt)
            junk = cpool.tile([128, 384], BF, tag="junk")
            nc.vector.memset(junk[:], 1.0)
            jps = pswpool.tile([128, 384], DT)
            for _ in range(16):
                nc.tensor.matmul(
                    jps[:], junk[:, :128], junk[:], start=True, stop=True
                )

            # --- rotating output buffers, zero slots pre-filled once ---
            otiles = []
            for i in range(NOB):
                O = opool.tile([128, RES * S27], DT, tag=f"ob{i}", bufs=1)
                O3 = O[:].rearrange("p (z s) -> p z s", s=S27)
                eng = nc.vector if i < 2 else nc.gpsimd
                eng.memset(O3[:, :, 5:8:2], 0.0)
                eng.memset(O3[:, :, 11:20:4], 0.0)
                eng.memset(O3[:, :, 21], 0.0)
                otiles.append(O)

            def h_chunk(b0, b1, z0=0, z1=RES):
                nb = b1 - b0
                zw = z1 - z0
                W = nb * zw
                assert nb == 1 or zw == RES
                if nb == 1:
                    csl = slice(RES * b0 + z0, RES * b0 + z1)
                else:
                    csl = slice(RES * b0, RES * b1)
                r2 = wpool.tile([128, 2 * nb * RES], DT, tag="r2", name="r2")[:, :2 * W]
                for n, eng in ((0, nc.gpsimd), (1, nc.vector)):
                    r2v = r2[:, n * W:(n + 1) * W].rearrange(
                        "p (b z) -> p b z", z=zw
                    )
                    crow = mi[:, M_CROW + RES * n + z0:M_CROW + RES * n + z1]
                    crow_b = _dc.replace(
                        crow, ap=[crow.ap[0], [0, nb], [1, zw]]
                    )
                    absl = mi[:, M_AB + EXTNT * n + b0:M_AB + EXTNT * n + b1]
                    ab_b = _dc.replace(absl, ap=[absl.ap[0], [1, nb], [0, zw]])
                    eng.tensor_add(r2v[:, :, :], crow_b, ab_b)
                s = wpool.tile([128, 2 * nb * RES], DT, tag="s", name="s")[:, :2 * W]
                nc.scalar.activation(s, r2, AF.Sqrt)
                sinv = wpool.tile([128, 2 * nb * RES], DT, tag="sinv", name="sinv")[:, :2 * W]
                nc.vector.reciprocal_approx_fast(sinv, s)
                psim = wpool.tile([128, nb * RES], DT, tag="psim", name="psim")[:, :W]
                nc.gpsimd.tensor_add(psim, sinv[:, :W], sinv[:, W:])
                nc.scalar.activation(HSQ[:, csl], psim, AF.Square, bias=1.0)
                nc.gpsimd.tensor_mul(Hb[:, csl], HSQ[:, csl], HSQ[:, csl])
                qc = wpool.tile([128, nb * RES], DT, tag="qc", name="qc")[:, :W]
                nc.vector.reciprocal_approx_fast(qc, HSQ[:, csl])
                nc.gpsimd.tensor_mul(HINV[:, csl], qc, qc)

            def do_tile(t, z0=0, z1=RES):
                zw = z1 - z0
                hsl = slice(RES * (t + 1) + z0, RES * (t + 1) + z1)
                p0 = pspool.tile([128, RES], DT, tag="p0", name="p0")[:, :zw]
                p1 = pspool.tile([128, RES], DT, tag="p1", name="p1")[:, :zw]
                for slot, pp in ((_g0_slot(t), p0), (_g1_slot(t), p1)):
                    for j in range(3):
                        lhs = dm[:, (slot * 3 + j) * 128:(slot * 3 + j + 1) * 128]
                        rsl = slice(RES * (t + j) + z0, RES * (t + j) + z1)
                        nc.tensor.matmul(
                            pp, lhs, Hb[:, rsl], start=(j == 0), stop=(j == 2)
                        )

                st = wopool.tile([128, RES], DT, tag="st", name="st")[:, :zw]
                Ht = Hb[:, RES * (t + 1):RES * (t + 2)]
                lo = 1 if z0 == 0 else 0
                hi = zw - 1 if z1 == RES else zw
                nc.gpsimd.tensor_sub(
                    st[:, lo:hi],
                    Ht[:, z0 + lo + 1:z0 + hi + 1],
                    Ht[:, z0 + lo - 1:z0 + hi - 1],
                )
                if z0 == 0:
                    nc.gpsimd.tensor_sub(st[:, 0:1], Ht[:, 1:2], Ht[:, 0:1])
                if z1 == RES:
                    nc.gpsimd.tensor_sub(
                        st[:, zw - 1:zw], Ht[:, 95:96], Ht[:, 94:95]
                    )
                stk = wopool.tile([128, RES], DT, tag="stk", name="stk")[:, :zw]
                nc.gpsimd.tensor_mul(stk, st, mi[:, M_KVEC + z0:M_KVEC + z1])
                # (z,c)-interleaved W row: W[z,c] = 0.5*G_c/h
                w3 = wopool.tile([128, 3 * RES], DT, tag="w3", name="w3")[:, :3 * zw]
                W3v = w3.rearrange("p (z c) -> p z c", c=3)
                nc.vector.tensor_mul(W3v[:, :, 0], p0, HINV[:, hsl])
                nc.vector.tensor_mul(W3v[:, :, 1], p1, HINV[:, hsl])
                nc.vector.tensor_mul(W3v[:, :, 2], stk, HINV[:, hsl])

                O = otiles[t % NOB]
                O3 = O[:].rearrange("p (z s) -> p z s", s=S27)
                Oz = O3[:, z0:z1, :]

                def wsrc(c, k):
                    ap_ = W3v[:, :, c]
                    return _dc.replace(ap_, ap=ap_.ap + [[0, k]])

                # 9 diagonal (i==j) slots s=12i+c in one packed-inner op
                dd = O3[:, z0:z1, 0]
                ds = W3v[:, :, 0]
                nc.scalar.copy(
                    _dc.replace(dd, ap=dd.ap + [[12, 3], [1, 3]]),
                    _dc.replace(ds, ap=ds.ap + [[0, 3], [1, 3]]),
                )
                # +W_c pairs (i==k): {10,20}->W0 DVE; {3,23}->W1, {6,16}->W2 Act
                nc.vector.tensor_copy(Oz[:, :, 10:21:10], wsrc(0, 2))
                nc.scalar.copy(Oz[:, :, 3:24:20], wsrc(1, 2))
                nc.scalar.copy(Oz[:, :, 6:17:10], wsrc(2, 2))
                # -W_c pairs (j==k): {4,8},{9,17} DVE; {18,22} Act
                nc.vector.tensor_scalar_mul(Oz[:, :, 4:9:4], wsrc(0, 2), -1.0)
                nc.vector.tensor_scalar_mul(Oz[:, :, 9:18:8], wsrc(1, 2), -1.0)
                nc.scalar.mul(Oz[:, :, 18:23:4], wsrc(2, 2), -1.0)

                nc.sync.dma_start(
                    d_out[128 * t:128 * (t + 1), S27 * z0:S27 * z1],
                    O[:, S27 * z0:S27 * z1],
                )

            # fill path: half-z chunks for blocks 0-2 and a half-z tile 0,
            # so the first output DMA issues as early as possible. After
            # that, each chunk is emitted one tile-group ahead of its
            # consumers so the 8-op h ladder's latency hides behind the
            # previous tiles' scatter + DMA.
            for b in range(3):
                h_chunk(b, b + 1, 0, 49)
            do_tile(0, 0, 48)
            for b in range(3):
                h_chunk(b, b + 1, 49, RES)
            do_tile(0, 48, RES)
            h_chunk(3, 5)
            do_tile(1)
            h_chunk(5, 7)
            do_tile(2)
            do_tile(3)
            h_chunk(7, 9)
            do_tile(4)
            do_tile(5)
            h_chunk(9, 11)
            do_tile(6)
            do_tile(7)
            do_tile(8)

    nc.finalize()
    return nc


_CACHE = {}


def _get_setup():
    if "nc" not in _CACHE:
        x, DX = _grid_x()
        _CACHE["x"] = x
        _CACHE["static"] = [_build_static(c, x, DX) for c in range(N_CORES)]
        _CACHE["nc"] = _build_program()
    return _CACHE["nc"], _CACHE["static"]


def _in_maps(BH_positions, BH_masses_presoftplus):
    nc, static = _get_setup()
    x = _CACHE["x"]
    pos = np.asarray(BH_positions, np.float64)
    pre = np.asarray(BH_masses_presoftplus, np.float64)
    mh = np.log1p(np.exp(pre)) * 0.5          # softplus(pre) / 2
    return [
        {"misc": _build_misc(static[c], x, pos, mh), "dmat": static[c]["dmat"]}
        for c in range(N_CORES)
    ]


def kernel(BH_positions, BH_masses_presoftplus):
    from concourse.bass_utils import run_bass_kernel_spmd

    nc, _ = _get_setup()
    in_maps = _in_maps(BH_positions, BH_masses_presoftplus)
    res = run_bass_kernel_spmd(nc, in_maps, list(range(N_CORES)))
    parts = [
        res.results[c]["out"].reshape(PLANES, RES, RES, 3, 3, 3)
        for c in range(N_CORES)
    ]
    return np.ascontiguousarray(np.concatenate(parts, axis=0))


# revision 33
# speedup vs baseline: 1.1315x; 1.1315x over previous
"""Trainium2 Bass kernel for the Brill-Lindquist Christoffel-symbol grid.

Math: the reference reduces to
    psi  = 1 + sum_n m_n / (2 r_n),   m = softplus(pre)
    h    = psi^4
    G_c  = finite-difference gradient of h along grid axis c (2nd order
           central interior, 1st order one-sided edges, spacing DX)
    W_c  = 0.5 * G_c / h
    Gamma^i_{jk} = delta_ij W_k + delta_ik W_j - delta_jk W_i
so the [96,96,96,3,3,3] output is +-W_c scattered over 27 slots per point.

Sharding: axis 0 (12 planes per core x 8 cores); h is analytic in the
inputs so each core evaluates its slab + 1-tile halo directly. Rows are
packed row = a0*96+a1 (9 tiles of 128 partitions), free dim = a2.

Design (output-DMA roofline ~29 us/core + prologue/fill):
  - h chain: s_n = r_n/mh_n via host-prescaled crow/ab broadcast adds
    (Pool+DVE into one [.,2W] tile), ONE Act sqrt over both BHs, ONE
    DVE reciprocal, psi-1 = sinv_lo+sinv_hi (Pool), hsq=(psi)^2 (Act,
    bias=1), h = hsq^2 bf16 (Pool); 1/h hoisted per-chunk (DVE recip +
    Pool square).
  - axis-0/1 FD: 6 bf16 matmuls/tile (fp32 psum), single-bf16 h
    (rel err 2.9e-3, budget 2e-2). z-FD via shifted subtract (Pool),
    kvec scale folded before the 1/h product.
  - scatter via the (z,c)-interleaved w3 row into the 27-slot output:
    9 diagonal slots in one packed-inner Act op; +/- pairs split
    DVE/Act at their measured-efficient op shapes (strided writes cost
    ~1.5-2 ns/elem on DVE/Act and are catastrophic on GpSimd, which
    also cannot read PSUM).
  - just-in-time h chunks emitted one tile-group ahead of their
    consumers; blocks 0-2 and tile 0 computed in z-halves so the first
    output DMA issues ~10 us earlier; PE pstate-ramp warmup matmuls;
    Act sqrt-table preload; 5 rotating output buffers; debug=False.
"""

import numpy as np

RES = 96
N_CORES = 8
PLANES = RES // N_CORES        # 12
LROWS = PLANES * RES           # 1152 local rows
NT = LROWS // 128              # 9 local 128-row tiles
EXTNT = NT + 2                 # 11 extended tiles (halo)
NROWS_G = RES * RES            # 9216 global rows
S27 = 27
NOB = 5                        # rotating output buffers

# misc input tile [128, MW] column layout (host-prescaled by 1/mh^2)
M_CROW = 0        # (z-pz_n)^2/mh_n^2 [96] x2
M_KVEC = 192      # z-FD column scale [96] (0.25/DX interior, 0.5/DX edge)
M_AB = 288        # ((x-px_n)^2+(y-py_n)^2)/mh_n^2 [11] x2
MW = 312

# dmat entry order: tile-0's two entries first so a small leading DMA
# unblocks the first tile's matmuls early.
ORDER = [0, 3, 1, 4, 5, 2]
SLOT = {e: i for i, e in enumerate(ORDER)}

# h-phase chunks (ext-block ranges): singles first for fill latency
CHUNKS = [(0, 1), (1, 2), (2, 3), (3, 5), (5, 7), (7, 9), (9, 11)]
TILES_AFTER_CHUNK = {2: [0], 3: [1, 2], 4: [3, 4], 5: [5, 6], 6: [7, 8]}


def _grid_x():
    # Match the reference grid bit-for-bit: jnp.linspace in fp32 on CPU
    # (the reference's softplus cannot compile for the neuron backend, so
    # it necessarily runs on the jax CPU platform).
    import jax
    import jax.numpy as jnp
    MAX_X = 1.0
    DX = np.float32(MAX_X / (RES / 2 - 1))

    def _ls():
        return jnp.linspace(
            DX * (1 - RES / 2), DX * (RES / 2 - 1), RES, dtype=jnp.float32
        )

    try:
        with jax.default_device(jax.devices("cpu")[0]):
            x = np.asarray(_ls())
    except Exception:
        x = np.asarray(_ls())
    return x, float(DX)


def _fd_sources(idx, coeff_c, coeff_e):
    """(offset, coeff) pairs for d/didx with 1st-order one-sided edges."""
    if idx == 0:
        return [(1, coeff_e), (0, -coeff_e)]
    if idx == RES - 1:
        return [(0, coeff_e), (-1, -coeff_e)]
    return [(1, coeff_c), (-1, -coeff_c)]


def _build_dmat(core, DX):
    """[128, 6*3*128] bf16 FD matrices as matmul lhsT ([q, p] = coeff of
    ext-row q in output row p); 0.5 Christoffel factor folded in. All
    values are +-0.25/DX or +-0.5/DX = +-11.75 / +-23.5, exact in bf16.
    Logical entries: 0 g0(t=0), 1 g0(interior), 2 g0(t=8), 3..5 g1(t%3);
    stored in column slots per ORDER."""
    import ml_dtypes
    c0 = 0.5 * (1.0 / (2.0 * np.float64(DX)))
    ce = 0.5 * (1.0 / np.float64(DX))
    out = np.zeros((128, 6 * 3 * 128), np.float64)

    def fill(entry, t, axis):
        slot = SLOT[entry]
        for p in range(128):
            gr = core * LROWS + 128 * t + p
            a = (gr // RES) if axis == 0 else (gr % RES)
            step = RES if axis == 0 else 1
            for off, cf in _fd_sources(a, c0, ce):
                g2 = gr + off * step
                e_ = g2 - core * LROWS + 128
                j = e_ // 128 - t
                q = e_ - 128 * (t + j)
                assert 0 <= j <= 2 and 0 <= q < 128, (core, t, p, off)
                out[q, (slot * 3 + j) * 128 + p] = cf

    fill(0, 0, 0)
    fill(1, 1, 0)
    fill(2, NT - 1, 0)
    for v in range(3):
        fill(3 + v, v, 1)
    return out.astype(ml_dtypes.bfloat16)


def _g0_slot(t):
    return SLOT[0] if t == 0 else (SLOT[2] if t == NT - 1 else SLOT[1])


def _g1_slot(t):
    return SLOT[3 + (t % 3)]


def _build_static(core, x, DX):
    slab = core * LROWS
    e = np.arange(EXTNT * 128)
    g = np.clip(slab - 128 + e, 0, NROWS_G - 1)   # clamp halo overrun (unused rows)
    xcol = x[g % RES].reshape(EXTNT, 128).T.copy()     # X coordinate (a1)
    ycol = x[g // RES].reshape(EXTNT, 128).T.copy()    # Y coordinate (a0)
    kvec = np.full(RES, 0.25 / DX, np.float64)
    kvec[0] = kvec[-1] = 0.5 / DX
    return {
        "xcol": np.ascontiguousarray(xcol, np.float64),
        "ycol": np.ascontiguousarray(ycol, np.float64),
        "kvec": kvec,
        "dmat": _build_dmat(core, DX),
    }


def _build_misc(static, x, pos, mh):
    """Per-core [128, MW] fp32 misc tile: prescaled crow/ab + kvec."""
    misc = np.zeros((128, MW), np.float64)
    for n in range(2):
        crow = (x.astype(np.float64) - pos[n, 2]) ** 2 / (mh[n] * mh[n])
        misc[:, M_CROW + RES * n:M_CROW + RES * (n + 1)] = crow[None, :]
        ab = ((static["xcol"] - pos[n, 0]) ** 2
              + (static["ycol"] - pos[n, 1]) ** 2) / (mh[n] * mh[n])
        misc[:, M_AB + EXTNT * n:M_AB + EXTNT * (n + 1)] = ab
    misc[:, M_KVEC:M_KVEC + RES] = static["kvec"][None, :]
    return np.ascontiguousarray(misc, np.float32)


def _build_program():
    import dataclasses as _dc

    import concourse.bacc as bacc
    import concourse.mybir as mybir
    import concourse.tile as tile

    DT = mybir.dt.float32
    BF = mybir.dt.bfloat16
    AF = mybir.ActivationFunctionType

    nc = bacc.Bacc(None, target_bir_lowering=False, debug=False)
    d_misc = nc.dram_tensor("misc", [128, MW], DT, kind="ExternalInput")
    d_dmat = nc.dram_tensor("dmat", [128, 6 * 3 * 128], BF, kind="ExternalInput")
    d_out = nc.dram_tensor("out", [LROWS, RES * S27], DT, kind="ExternalOutput")

    HW_ = EXTNT * RES             # 1056: free width of the ext h field
    with tile.TileContext(nc) as tc:
        with (
            tc.tile_pool(name="const", bufs=1) as cpool,
            tc.tile_pool(name="work", bufs=4) as wpool,
            tc.tile_pool(name="wout", bufs=4) as wopool,
            tc.tile_pool(name="obuf", bufs=1) as opool,
            tc.tile_pool(name="psum", bufs=3, space="PSUM") as pspool,
            tc.tile_pool(name="psw", bufs=1, space="PSUM") as pswpool,
        ):
            # --- inputs in: misc first (unblocks the h chain), dmat in
            # two pieces (tile-0's entries lead) ---
            mi = cpool.tile([128, MW], DT)
            nc.sync.dma_start(mi[:], d_misc[:])
            dm = cpool.tile([128, 6 * 3 * 128], BF)
            nc.sync.dma_start(dm[:, :6 * 128], d_dmat[:, :6 * 128])
            nc.sync.dma_start(dm[:, 6 * 128:], d_dmat[:, 6 * 128:])

            # --- persistent fields ---
            HSQ = cpool.tile([128, HW_], DT)   # psi^2, fp32
            Hb = cpool.tile([128, HW_], BF)    # h = psi^4, bf16 (FD source)
            HINV = cpool.tile([128, HW_], DT)  # 1/h

            # --- warmups: Act sqrt-table preload (dep-free garbage read)
            # + PE pstate ramp, both overlap the input DMAs ---
            jact = cpool.tile([1, 8], DT, tag="jact")
            nc.vector.memset(jact[:], 1.0)
            nc.scalar.activation(jact[:], jact[:], AF.Sqrt)
            junk = cpool.tile([128, 384], BF, tag="junk")
            nc.vector.memset(junk[:], 1.0)
            jps = pswpool.tile([128, 384], DT)
            for _ in range(16):
                nc.tensor.matmul(
                    jps[:], junk[:, :128], junk[:], start=True, stop=True
                )

            # --- rotating output buffers; the zero-slot memsets are
            # emitted after the fill-critical chunk ops (below) so they
            # don't delay DVE/Pool's first h-ladder work ---
            otiles = [
                opool.tile([128, RES * S27], DT, tag=f"ob{i}", bufs=1, name=f"ob{i}")
                for i in range(NOB)
            ]

            def zero_slots(i):
                O3 = otiles[i][:].rearrange("p (z s) -> p z s", s=S27)
                eng = nc.vector if i < 2 else nc.gpsimd
                eng.memset(O3[:, :, 5:8:2], 0.0)
                eng.memset(O3[:, :, 11:20:4], 0.0)
                eng.memset(O3[:, :, 21], 0.0)

            def h_chunk(b0, b1, z0=0, z1=RES):
                nb = b1 - b0
                zw = z1 - z0
                W = nb * zw
                assert nb == 1 or zw == RES
                if nb == 1:
                    csl = slice(RES * b0 + z0, RES * b0 + z1)
                else:
                    csl = slice(RES * b0, RES * b1)
                r2 = wpool.tile([128, 2 * nb * RES], DT, tag="r2", name="r2")[:, :2 * W]
                for n, eng in ((0, nc.gpsimd), (1, nc.vector)):
                    r2v = r2[:, n * W:(n + 1) * W].rearrange(
                        "p (b z) -> p b z", z=zw
                    )
                    crow = mi[:, M_CROW + RES * n + z0:M_CROW + RES * n + z1]
                    crow_b = _dc.replace(
                        crow, ap=[crow.ap[0], [0, nb], [1, zw]]
                    )
                    absl = mi[:, M_AB + EXTNT * n + b0:M_AB + EXTNT * n + b1]
                    ab_b = _dc.replace(absl, ap=[absl.ap[0], [1, nb], [0, zw]])
                    eng.tensor_add(r2v[:, :, :], crow_b, ab_b)
                s = wpool.tile([128, 2 * nb * RES], DT, tag="s", name="s")[:, :2 * W]
                nc.scalar.activation(s, r2, AF.Sqrt)
                sinv = wpool.tile([128, 2 * nb * RES], DT, tag="sinv", name="sinv")[:, :2 * W]
                nc.vector.reciprocal_approx_fast(sinv, s)
                psim = wpool.tile([128, nb * RES], DT, tag="psim", name="psim")[:, :W]
                nc.gpsimd.tensor_add(psim, sinv[:, :W], sinv[:, W:])
                nc.scalar.activation(HSQ[:, csl], psim, AF.Square, bias=1.0)
                nc.gpsimd.tensor_mul(Hb[:, csl], HSQ[:, csl], HSQ[:, csl])
                qc = wpool.tile([128, nb * RES], DT, tag="qc", name="qc")[:, :W]
                nc.vector.reciprocal_approx_fast(qc, HSQ[:, csl])
                nc.gpsimd.tensor_mul(HINV[:, csl], qc, qc)

            def do_tile(t, z0=0, z1=RES):
                zw = z1 - z0
                hsl = slice(RES * (t + 1) + z0, RES * (t + 1) + z1)
                p0 = pspool.tile([128, RES], DT, tag="p0", name="p0")[:, :zw]
                p1 = pspool.tile([128, RES], DT, tag="p1", name="p1")[:, :zw]
                for slot, pp in ((_g0_slot(t), p0), (_g1_slot(t), p1)):
                    for j in range(3):
                        lhs = dm[:, (slot * 3 + j) * 128:(slot * 3 + j + 1) * 128]
                        rsl = slice(RES * (t + j) + z0, RES * (t + j) + z1)
                        nc.tensor.matmul(
                            pp, lhs, Hb[:, rsl], start=(j == 0), stop=(j == 2)
                        )

                st = wopool.tile([128, RES], DT, tag="st", name="st")[:, :zw]
                Ht = Hb[:, RES * (t + 1):RES * (t + 2)]
                lo = 1 if z0 == 0 else 0
                hi = zw - 1 if z1 == RES else zw
                nc.gpsimd.tensor_sub(
                    st[:, lo:hi],
                    Ht[:, z0 + lo + 1:z0 + hi + 1],
                    Ht[:, z0 + lo - 1:z0 + hi - 1],
                )
                if z0 == 0:
                    nc.gpsimd.tensor_sub(st[:, 0:1], Ht[:, 1:2], Ht[:, 0:1])
                if z1 == RES:
                    nc.gpsimd.tensor_sub(
                        st[:, zw - 1:zw], Ht[:, 95:96], Ht[:, 94:95]
                    )
                stk = wopool.tile([128, RES], DT, tag="stk", name="stk")[:, :zw]
                nc.gpsimd.tensor_mul(stk, st, mi[:, M_KVEC + z0:M_KVEC + z1])
                # (z,c)-interleaved W row: W[z,c] = 0.5*G_c/h
                w3 = wopool.tile([128, 3 * RES], DT, tag="w3", name="w3")[:, :3 * zw]
                W3v = w3.rearrange("p (z c) -> p z c", c=3)
                nc.vector.tensor_mul(W3v[:, :, 0], p0, HINV[:, hsl])
                nc.vector.tensor_mul(W3v[:, :, 1], p1, HINV[:, hsl])
                nc.vector.tensor_mul(W3v[:, :, 2], stk, HINV[:, hsl])

                O = otiles[t % NOB]
                O3 = O[:].rearrange("p (z s) -> p z s", s=S27)
                Oz = O3[:, z0:z1, :]

                def wsrc(c, k):
                    ap_ = W3v[:, :, c]
                    return _dc.replace(ap_, ap=ap_.ap + [[0, k]])

                # 9 diagonal (i==j) slots s=12i+c in one packed-inner op
                dd = O3[:, z0:z1, 0]
                ds = W3v[:, :, 0]
                nc.scalar.copy(
                    _dc.replace(dd, ap=dd.ap + [[12, 3], [1, 3]]),
                    _dc.replace(ds, ap=ds.ap + [[0, 3], [1, 3]]),
                )
                # +W_c pairs (i==k): {10,20}->W0 DVE; {3,23}->W1, {6,16}->W2 Act
                nc.vector.tensor_copy(Oz[:, :, 10:21:10], wsrc(0, 2))
                nc.scalar.copy(Oz[:, :, 3:24:20], wsrc(1, 2))
                nc.scalar.copy(Oz[:, :, 6:17:10], wsrc(2, 2))
                # -W_c pairs (j==k): {4,8},{9,17} DVE; {18,22} Act
                nc.vector.tensor_scalar_mul(Oz[:, :, 4:9:4], wsrc(0, 2), -1.0)
                nc.vector.tensor_scalar_mul(Oz[:, :, 9:18:8], wsrc(1, 2), -1.0)
                nc.scalar.mul(Oz[:, :, 18:23:4], wsrc(2, 2), -1.0)

                nc.sync.dma_start(
                    d_out[128 * t:128 * (t + 1), S27 * z0:S27 * z1],
                    O[:, S27 * z0:S27 * z1],
                )

            # fill path: half-z chunks for blocks 0-2 and a half-z tile 0,
            # so the first output DMA issues as early as possible. After
            # that, each chunk is emitted one tile-group ahead of its
            # consumers so the 8-op h ladder's latency hides behind the
            # previous tiles' scatter + DMA.
            for b in range(3):
                h_chunk(b, b + 1, 0, 49)
            zero_slots(0)
            do_tile(0, 0, 48)
            for b in range(3):
                h_chunk(b, b + 1, 49, RES)
            zero_slots(1)
            do_tile(0, 48, RES)
            for i in range(2, NOB):
                zero_slots(i)
            h_chunk(3, 5)
            do_tile(1)
            h_chunk(5, 7)
            do_tile(2)
            do_tile(3)
            h_chunk(7, 9)
            do_tile(4)
            do_tile(5)
            h_chunk(9, 11)
            do_tile(6)
            do_tile(7)
            do_tile(8)

    nc.finalize()
    return nc


_CACHE = {}


def _get_setup():
    if "nc" not in _CACHE:
        x, DX = _grid_x()
        _CACHE["x"] = x
        _CACHE["static"] = [_build_static(c, x, DX) for c in range(N_CORES)]
        _CACHE["nc"] = _build_program()
    return _CACHE["nc"], _CACHE["static"]


def _in_maps(BH_positions, BH_masses_presoftplus):
    nc, static = _get_setup()
    x = _CACHE["x"]
    pos = np.asarray(BH_positions, np.float64)
    pre = np.asarray(BH_masses_presoftplus, np.float64)
    mh = np.log1p(np.exp(pre)) * 0.5          # softplus(pre) / 2
    return [
        {"misc": _build_misc(static[c], x, pos, mh), "dmat": static[c]["dmat"]}
        for c in range(N_CORES)
    ]


def kernel(BH_positions, BH_masses_presoftplus):
    from concourse.bass_utils import run_bass_kernel_spmd

    nc, _ = _get_setup()
    in_maps = _in_maps(BH_positions, BH_masses_presoftplus)
    res = run_bass_kernel_spmd(nc, in_maps, list(range(N_CORES)))
    parts = [
        res.results[c]["out"].reshape(PLANES, RES, RES, 3, 3, 3)
        for c in range(N_CORES)
    ]
    return np.ascontiguousarray(np.concatenate(parts, axis=0))
